# revision 9
# baseline (speedup 1.0000x reference)
"""GAE (generalized advantage estimation) Trainium2 kernel.

Problem: nn_CustomGAE — B=512, T=2048, D=64.
  value = obs @ W + b ; next_value = next_obs @ W + b
  td0 = reward + gamma*nd*next_value - value ; coef = gamma*lambda*nd
  A_t = td0_t + coef_t * A_{t+1}  (reverse scan over T, independent per trajectory)
  returns (advantage, value_target = advantage + value)

Sharding: pure data parallel over B across 8 cores (64 trajectories/core).

Per-core layout: the host pre-swizzles each 64-trajectory shard to
(half, batch)-major, so SBUF partition p = h*64 + b holds timesteps
t in [h*1024, (h+1)*1024) at a uniform DRAM stride — every streamed chunk is
one 128-partition dma_start with contiguous per-partition runs.

v2 pipeline (default):
  - obs streams on the SP HWDGE ring, next_obs on the Activation ring, so
    neither stream serializes behind the other; outputs go on the DVE ring.
  - DVE does obs*W + v-reduce; Pool (gpsimd) does next_obs*W + nv-reduce,
    so no single engine exceeds the DMA-engine-pool time per chunk.
  - Chunks stream in REVERSE time order with descending sizes at the end.
    Each chunk's td0 is computed on arrival (fused scalar_tensor_tensor) and
    scanned immediately, chaining `initial` across chunks. The late-time half
    (partitions 64..127) is exact and its outputs stream out per chunk; the
    early-time half scans with initial=0 while also scanning the running
    coef product P, and is fixed up at the very end as A = A0 + P*bnd
    (bnd = carry across the half boundary, one tiny SBUF->SBUF DMA).
    This leaves only a few microseconds of serial tail after the last byte
    arrives instead of the full epilogue+scan+writeback.
"""

import sys

sys.path.insert(0, "/opt/trn_rl_repo")

from contextlib import ExitStack

import numpy as np

import concourse.bacc as bacc
import concourse.mybir as mybir
import concourse.tile as tile
from concourse.bass_utils import run_bass_kernel_spmd

GAMMA = 0.99
LMBDA = 0.95

B, T, D = 512, 2048, 64
NCORES = 8
BL = B // NCORES  # 64 trajectories per core
H = 2  # trajectory halves stacked on partitions -> 128 partitions
P = H * BL  # 128
F32 = mybir.dt.float32
U8 = mybir.dt.uint8

# Results of the last hardware run, for test harnesses.
LAST_RESULTS = None


def _rsl(off, sz):
    """Reversed free-dim slice covering columns [off, off+sz)."""
    return slice(off + sz - 1, (off - 1) if off > 0 else None, -1)


def _build_iter2(
    nc, opool, npool, qpool, dpool, w_t, b_t,
    obs_d, nobs_d, rw_d, dn_d, adv_d, tgt_d, tp, sizes,
    nv_on="pool",
):
    """One full pass, reverse-streamed with incremental blocked scan."""
    mult = mybir.AluOpType.mult
    add = mybir.AluOpType.add
    sub = mybir.AluOpType.subtract
    bypass = mybir.AluOpType.bypass
    X = mybir.AxisListType.X
    oeng = nc.scalar  # output DMAs ride the Activation ring between inputs

    rw_t = dpool.tile([P, tp], F32)
    dn_t = dpool.tile([P, tp], U8)
    nc.sync.dma_start(rw_t[:], rw_d.ap())
    nc.sync.dma_start(dn_t[:], dn_d.ap())

    # done -> decay factors, on Pool so the DMA rings stay compute-free.
    ndf = dpool.tile([P, tp], F32)
    nc.gpsimd.tensor_copy(ndf[:], dn_t[:])  # u8 -> f32
    g = dpool.tile([P, tp], F32)  # gamma * nd
    nc.gpsimd.tensor_scalar(g[:], ndf[:], -GAMMA, GAMMA, mult, add)
    coef = dpool.tile([P, tp], F32)  # gamma * lambda * nd
    nc.gpsimd.tensor_scalar(coef[:], ndf[:], -GAMMA * LMBDA, GAMMA * LMBDA, mult, add)

    v_raw = dpool.tile([P, tp], F32)  # obs @ W (no bias)
    adv = dpool.tile([P, tp], F32)
    Pt = dpool.tile([P, tp], F32)  # running suffix product of coef
    tgt = dpool.tile([P, tp], F32)
    bnd = dpool.tile([BL, 1], F32)

    hi = slice(BL, 2 * BL)
    lo = slice(0, BL)
    bulk = max(sizes)
    wb = {
        sz: w_t[:].unsqueeze(1).broadcast_to([P, sz, D])
        for sz in sorted(set(sizes))
    }

    off = tp
    prev_off = None  # column of the previously scanned chunk's first element
    out_hwm = tp  # columns [out_hwm, tp) already written back (hi half)
    for j, sz in enumerate(sizes):
        off -= sz
        cs = slice(off, off + sz)
        fs = slice(off * D, (off + sz) * D)
        ot = opool.tile([P, bulk * D], F32)
        nt = npool.tile([P, bulk * D], F32)
        nc.sync.dma_start(ot[:, : sz * D], obs_d.ap()[:, fs])
        nc.scalar.dma_start(nt[:, : sz * D], nobs_d.ap()[:, fs])
        o3 = ot[:, : sz * D].rearrange("p (t d) -> p t d", d=D)
        n3 = nt[:, : sz * D].rearrange("p (t d) -> p t d", d=D)

        # value head: both mults on Pool (free-axis reduces are DVE-only)
        meng = nc.gpsimd if nv_on == "pool" else nc.vector
        meng.tensor_tensor(out=o3, in0=o3, in1=wb[sz], op=mult)
        nc.vector.tensor_reduce(out=v_raw[:, cs], in_=o3, axis=X, op=add)
        nvs = qpool.tile([P, bulk], F32)
        meng.tensor_tensor(out=n3, in0=n3, in1=wb[sz], op=mult)
        nc.vector.tensor_reduce(out=nvs[:, :sz], in_=n3, axis=X, op=add)

        # td0 = (nv + b)*g - ((v + b) - rw), fused scalar_tensor_tensor.
        q = qpool.tile([P, bulk], F32)
        nc.vector.scalar_tensor_tensor(
            out=q[:, :sz], in0=nvs[:, :sz], scalar=b_t[:, 0:1], in1=g[:, cs],
            op0=add, op1=mult,
        )
        t1 = qpool.tile([P, bulk], F32)
        nc.vector.scalar_tensor_tensor(
            out=t1[:, :sz], in0=v_raw[:, cs], scalar=b_t[:, 0:1], in1=rw_t[:, cs],
            op0=add, op1=sub,
        )
        nc.vector.tensor_tensor(out=q[:, :sz], in0=q[:, :sz], in1=t1[:, :sz], op=sub)

        # incremental reverse scans, chained via `initial`
        rs = _rsl(off, sz)
        qrs = slice(sz - 1, None, -1)
        a_init = 0.0 if prev_off is None else adv[:, prev_off : prev_off + 1]
        nc.vector.tensor_tensor_scan(
            out=adv[:, rs], data0=coef[:, rs], data1=q[:, qrs],
            initial=a_init, op0=mult, op1=add,
        )
        p_init = 1.0 if prev_off is None else Pt[:, prev_off : prev_off + 1]
        nc.vector.tensor_tensor_scan(
            out=Pt[:, rs], data0=coef[:, rs], data1=coef[:, rs],
            initial=p_init, op0=mult, op1=bypass,
        )
        prev_off = off

        # late-time half is exact: finalize per chunk, stream out in batches
        nc.vector.scalar_tensor_tensor(
            out=tgt[hi, cs], in0=adv[hi, cs], scalar=b_t[BL:, 0:1],
            in1=v_raw[hi, cs], op0=add, op1=add,
        )
        if out_hwm - off >= 192 or j == len(sizes) - 1:
            ob = slice(off, out_hwm)
            oeng.dma_start(adv_d.ap()[hi, ob], adv[hi, ob])
            oeng.dma_start(tgt_d.ap()[hi, ob], tgt[hi, ob])
            out_hwm = off

    # tail: carry the boundary value across halves, fix up the early half.
    # Done in two column halves so the writeback DMA of the first overlaps
    # the fixup compute of the second.
    nc.sync.dma_start(bnd[:], adv[hi, 0:1])
    half = tp // 2
    for c0, c1 in ((0, half), (half, tp)):
        cs = slice(c0, c1)
        nc.vector.scalar_tensor_tensor(
            out=adv[lo, cs], in0=Pt[lo, cs], scalar=bnd[:, 0:1],
            in1=adv[lo, cs], op0=mult, op1=add,
        )
        nc.vector.scalar_tensor_tensor(
            out=tgt[lo, cs], in0=adv[lo, cs], scalar=b_t[:BL, 0:1],
            in1=v_raw[lo, cs], op0=add, op1=add,
        )
        nc.sync.dma_start(adv_d.ap()[lo, cs], adv[lo, cs])
        nc.scalar.dma_start(tgt_d.ap()[lo, cs], tgt[lo, cs])


def build_program(
    t_total=T, bulk=64, tail_sizes=(32, 16, 8, 8), repeat=1,
    nv_on="pool", bufs=3, dbl=2, bench_internal=False,
):
    """Build the per-core Bass program (all 8 cores run it SPMD on their own
    shard). DRAM tensor layouts are (half, batch)-major as produced by
    shard_inputs. repeat>1 re-runs the whole pipeline inside one NEFF
    (test.py uses the delta vs repeat=1 to measure per-iteration HW time).
    bench_internal makes obs/next_obs Internal DRAM (not shipped per call;
    garbage values) so benchmark calls are cheap — timing-only builds."""
    tp = t_total // H  # timesteps per partition
    rest = tp - sum(tail_sizes)
    assert rest >= 0 and rest % bulk == 0
    sizes = [bulk] * (rest // bulk) + list(tail_sizes)
    assert sum(sizes) == tp

    nc = bacc.Bacc(
        "TRN2", target_bir_lowering=False, debug=False, enable_asserts=False
    )

    big_kind = "Internal" if bench_internal else "ExternalInput"
    obs_d = nc.dram_tensor("obs", [P, tp * D], F32, kind=big_kind)
    nobs_d = nc.dram_tensor("nobs", [P, tp * D], F32, kind=big_kind)
    rw_d = nc.dram_tensor("rw", [P, tp], F32, kind="ExternalInput")
    dn_d = nc.dram_tensor("dn", [P, tp], U8, kind="ExternalInput")
    w_d = nc.dram_tensor("w", [D], F32, kind="ExternalInput")
    b_d = nc.dram_tensor("b", [1], F32, kind="ExternalInput")
    adv_d = nc.dram_tensor("adv", [P, tp], F32, kind="ExternalOutput")
    tgt_d = nc.dram_tensor("tgt", [P, tp], F32, kind="ExternalOutput")

    with tile.TileContext(nc) as tc, ExitStack() as ctx:
        cpool = ctx.enter_context(tc.tile_pool(name="const", bufs=1))
        opool = ctx.enter_context(tc.tile_pool(name="obs", bufs=bufs))
        npool = ctx.enter_context(tc.tile_pool(name="nobs", bufs=bufs))
        qpool = ctx.enter_context(tc.tile_pool(name="chunk", bufs=3))
        dpool = ctx.enter_context(tc.tile_pool(name="iter", bufs=dbl))

        # Value-head weights replicated to every partition.
        w_t = cpool.tile([P, D], F32)
        nc.sync.dma_start(w_t[:], w_d.ap().unsqueeze(0).broadcast_to([P, D]))
        b_t = cpool.tile([P, 1], F32)
        nc.sync.dma_start(b_t[:], b_d.ap().unsqueeze(0).broadcast_to([P, 1]))

        for _rep in range(repeat):
            _build_iter2(
                nc, opool, npool, qpool, dpool, w_t, b_t,
                obs_d, nobs_d, rw_d, dn_d, adv_d, tgt_d, tp, sizes,
                nv_on=nv_on,
            )

    # Runs the bacc pipeline (register allocation etc.) — required before
    # serializing for the walrus compiler.
    nc.finalize()
    return nc


_NC_CACHE = None


def _get_nc():
    global _NC_CACHE
    if _NC_CACHE is None:
        _NC_CACHE = build_program()
    return _NC_CACHE


def _hmajor(x, tp_cols):
    """[BL, H*tp_cols] row-major -> [H*BL, tp_cols] with row p = h*BL + b."""
    return np.ascontiguousarray(
        x.reshape(BL, H, tp_cols).transpose(1, 0, 2).reshape(H * BL, tp_cols)
    )


def _unhmajor(y):
    """Inverse of _hmajor for outputs: [H*BL, tp] -> [BL, H*tp]."""
    tp = y.shape[1]
    return y.reshape(H, BL, tp).transpose(1, 0, 2).reshape(BL, H * tp)


def shard_inputs(obs, next_obs, reward, done, W, b):
    """Split full inputs into the 8 per-core input maps ((h,b)-major)."""
    obs = np.asarray(obs, dtype=np.float32).reshape(B, T * D)
    nobs = np.asarray(next_obs, dtype=np.float32).reshape(B, T * D)
    rw = np.asarray(reward, dtype=np.float32).reshape(B, T)
    dn = np.asarray(done).astype(np.uint8, copy=False).reshape(B, T)
    w_np = np.ascontiguousarray(np.asarray(W, dtype=np.float32)).reshape(D)
    b_np = np.ascontiguousarray(np.asarray(b, dtype=np.float32)).reshape(1)

    tpd = (T // H) * D
    tp = T // H
    in_maps = []
    for i in range(NCORES):
        sl = slice(i * BL, (i + 1) * BL)
        in_maps.append(
            {
                "obs": _hmajor(obs[sl], tpd),
                "nobs": _hmajor(nobs[sl], tpd),
                "rw": _hmajor(rw[sl], tp),
                "dn": _hmajor(dn[sl], tp),
                "w": w_np,
                "b": b_np,
            }
        )
    return in_maps


def gather_outputs(results):
    advantage = np.concatenate(
        [_unhmajor(r["adv"]) for r in results], axis=0
    ).reshape(B, T, 1)
    value_target = np.concatenate(
        [_unhmajor(r["tgt"]) for r in results], axis=0
    ).reshape(B, T, 1)
    return advantage, value_target


def kernel(obs, next_obs, reward, done, W, b):
    global LAST_RESULTS
    nc = _get_nc()
    in_maps = shard_inputs(obs, next_obs, reward, done, W, b)
    res = run_bass_kernel_spmd(nc, in_maps, core_ids=list(range(NCORES)))
    LAST_RESULTS = res
    return gather_outputs(res.results)


# revision 14
# speedup vs baseline: 2.2985x; 2.2985x over previous
"""GAE (generalized advantage estimation) Trainium2 kernel.

Problem: nn_CustomGAE — B=512, T=2048, D=64.
  value = obs @ W + b ; next_value = next_obs @ W + b
  td0 = reward + gamma*nd*next_value - value ; coef = gamma*lambda*nd
  A_t = td0_t + coef_t * A_{t+1}  (reverse scan over T, independent per trajectory)
  returns (advantage, value_target = advantage + value)

Sharding: pure data parallel over B across 8 cores (64 trajectories/core).

Per-core layout: the host pre-swizzles each 64-trajectory shard to
(half, batch)-major, so SBUF partition p = h*64 + b holds timesteps
t in [h*1024, (h+1)*1024) at a uniform DRAM stride — every streamed chunk is
one 128-partition dma_start with contiguous per-partition runs.

v2 pipeline (default):
  - obs streams on the SP HWDGE ring, next_obs on the Activation ring, so
    neither stream serializes behind the other; outputs go on the DVE ring.
  - DVE does obs*W + v-reduce; Pool (gpsimd) does next_obs*W + nv-reduce,
    so no single engine exceeds the DMA-engine-pool time per chunk.
  - Chunks stream in REVERSE time order with descending sizes at the end.
    Each chunk's td0 is computed on arrival (fused scalar_tensor_tensor) and
    scanned immediately, chaining `initial` across chunks. The late-time half
    (partitions 64..127) is exact and its outputs stream out per chunk; the
    early-time half scans with initial=0 while also scanning the running
    coef product P, and is fixed up at the very end as A = A0 + P*bnd
    (bnd = carry across the half boundary, one tiny SBUF->SBUF DMA).
    This leaves only a few microseconds of serial tail after the last byte
    arrives instead of the full epilogue+scan+writeback.
"""

import sys

sys.path.insert(0, "/opt/trn_rl_repo")

from contextlib import ExitStack

import numpy as np

import concourse.bacc as bacc
import concourse.mybir as mybir
import concourse.tile as tile
import ml_dtypes
from concourse.bass_utils import run_bass_kernel_spmd

GAMMA = 0.99
LMBDA = 0.95

B, T, D = 512, 2048, 64
NCORES = 8
BL = B // NCORES  # 64 trajectories per core
H = 2  # trajectory halves stacked on partitions -> 128 partitions
P = H * BL  # 128
F32 = mybir.dt.float32
BF16 = mybir.dt.bfloat16
U8 = mybir.dt.uint8

# obs/next_obs stream dtype: host casts to bf16 (value head is far inside the
# 2e-2 tolerance) which halves stream DMA and enables DVE packed-dtype modes.
IN_DT = "bf16"
IN_NP = ml_dtypes.bfloat16 if IN_DT == "bf16" else np.float32

# Results of the last hardware run, for test harnesses.
LAST_RESULTS = None


def _rsl(off, sz):
    """Reversed free-dim slice covering columns [off, off+sz)."""
    return slice(off + sz - 1, (off - 1) if off > 0 else None, -1)


def _build_iter2(
    nc, opool, npool, qpool, dpool, w_t, b_t,
    obs_d, nobs_d, rw_d, dn_d, adv_d, tgt_d, tp, sizes,
    sdt, red_mode="direct", nv_on="dve", small_on="pool",
):
    """One full pass, reverse-streamed with incremental blocked scan."""
    mult = mybir.AluOpType.mult
    add = mybir.AluOpType.add
    sub = mybir.AluOpType.subtract
    bypass = mybir.AluOpType.bypass
    X = mybir.AxisListType.X
    oeng = nc.scalar  # output DMAs ride the Activation ring between inputs

    rw_t = dpool.tile([P, tp], F32)
    dn_t = dpool.tile([P, tp], U8)
    nc.sync.dma_start(rw_t[:], rw_d.ap())
    nc.sync.dma_start(dn_t[:], dn_d.ap())

    # done -> decay factors, on Pool so the DMA rings stay compute-free.
    ndf = dpool.tile([P, tp], F32)
    nc.gpsimd.tensor_copy(ndf[:], dn_t[:])  # u8 -> f32
    g = dpool.tile([P, tp], F32)  # gamma * nd
    nc.gpsimd.tensor_scalar(g[:], ndf[:], -GAMMA, GAMMA, mult, add)
    coef = dpool.tile([P, tp], F32)  # gamma * lambda * nd
    nc.gpsimd.tensor_scalar(coef[:], ndf[:], -GAMMA * LMBDA, GAMMA * LMBDA, mult, add)
    # rw2 = rw + b*(g-1) folds the value-head bias out of the per-chunk path:
    # td0 = rw + g*(nv+b) - (v+b) = (g*nv - v) + rw2, all plain tensor_tensor.
    gm1 = dpool.tile([P, tp], F32)
    nc.gpsimd.tensor_scalar(gm1[:], ndf[:], -GAMMA, GAMMA - 1.0, mult, add)
    rw2 = dpool.tile([P, tp], F32)
    nc.vector.scalar_tensor_tensor(
        out=rw2[:], in0=gm1[:], scalar=b_t[:, 0:1], in1=rw_t[:], op0=mult, op1=add,
    )

    vdt = sdt if red_mode == "direct" else F32
    v_raw = dpool.tile([P, tp], vdt)  # obs @ W (no bias)
    fold_w = 16  # tree modes: TT-fold D down to this, then one TensorReduce
    adv = dpool.tile([P, tp], F32)
    Pt = dpool.tile([P, tp], F32)  # running suffix product of coef
    tgt = dpool.tile([P, tp], F32)
    bnd = dpool.tile([BL, 1], F32)

    hi = slice(BL, 2 * BL)
    lo = slice(0, BL)
    bulk = max(sizes)
    wb = {
        sz: w_t[:].unsqueeze(1).broadcast_to([P, sz, D])
        for sz in sorted(set(sizes))
    }

    off = tp
    prev_off = None  # column of the previously scanned chunk's first element
    out_hwm = tp  # columns [out_hwm, tp) already written back (hi half)
    for j, sz in enumerate(sizes):
        off -= sz
        cs = slice(off, off + sz)
        fs = slice(off * D, (off + sz) * D)
        ot = opool.tile([P, bulk * D], sdt)
        nt = npool.tile([P, bulk * D], sdt)
        nc.sync.dma_start(ot[:, : sz * D], obs_d.ap()[:, fs])
        nc.scalar.dma_start(nt[:, : sz * D], nobs_d.ap()[:, fs])
        o3 = ot[:, : sz * D].rearrange("p (t d) -> p t d", d=D)
        n3 = nt[:, : sz * D].rearrange("p (t d) -> p t d", d=D)

        # value head: in-place mult, then reduce over D (free-axis reduces
        # are DVE-only; Pool is ~3x slower per element on this HW)
        meng = nc.gpsimd if nv_on == "pool" else nc.vector
        seng = nc.gpsimd if small_on == "pool" else nc.vector
        nfold = {"ptree": 1, "p2tree": 2}.get(red_mode, 0)
        nvs = qpool.tile([P, bulk], vdt)
        for si, (x3, red_out) in enumerate(((o3, v_raw[:, cs]), (n3, nvs[:, :sz]))):
            meng.tensor_tensor(out=x3, in0=x3, in1=wb[sz], op=mult)
            if red_mode == "direct":
                nc.vector.tensor_reduce(out=red_out, in_=x3, axis=X, op=add)
            else:
                # halving TT folds run in the 2-byte packed fast mode (the
                # reduce does not), then one TensorReduce over the remnant
                w2 = D // 2
                while w2 >= fold_w:
                    # ptree/p2tree: first fold of obs (and nobs) goes to Pool
                    feng = (
                        nc.gpsimd
                        if (si < nfold and w2 == D // 2)
                        else nc.vector
                    )
                    feng.tensor_tensor(
                        out=x3[:, :, :w2], in0=x3[:, :, :w2],
                        in1=x3[:, :, w2 : 2 * w2], op=add,
                    )
                    w2 //= 2
                nc.vector.tensor_reduce(
                    out=red_out, in_=x3[:, :, : 2 * w2], axis=X, op=add
                )

        # td0 = g*nv - v + rw2, all Pool-eligible tensor_tensor ops
        q = qpool.tile([P, bulk], F32)
        seng.tensor_tensor(out=q[:, :sz], in0=g[:, cs], in1=nvs[:, :sz], op=mult)
        t1 = qpool.tile([P, bulk], F32)
        seng.tensor_tensor(out=t1[:, :sz], in0=rw2[:, cs], in1=v_raw[:, cs], op=sub)
        seng.tensor_tensor(out=q[:, :sz], in0=q[:, :sz], in1=t1[:, :sz], op=add)

        # incremental reverse scans, chained via `initial`
        rs = _rsl(off, sz)
        qrs = slice(sz - 1, None, -1)
        a_init = 0.0 if prev_off is None else adv[:, prev_off : prev_off + 1]
        nc.vector.tensor_tensor_scan(
            out=adv[:, rs], data0=coef[:, rs], data1=q[:, qrs],
            initial=a_init, op0=mult, op1=add,
        )
        p_init = 1.0 if prev_off is None else Pt[:, prev_off : prev_off + 1]
        nc.vector.tensor_tensor_scan(
            out=Pt[:, rs], data0=coef[:, rs], data1=coef[:, rs],
            initial=p_init, op0=mult, op1=bypass,
        )
        prev_off = off

        # late-time half is exact: finalize per chunk, stream out in batches
        nc.vector.scalar_tensor_tensor(
            out=tgt[hi, cs], in0=adv[hi, cs], scalar=b_t[BL:, 0:1],
            in1=v_raw[hi, cs], op0=add, op1=add,
        )
        if out_hwm - off >= 192 or j == len(sizes) - 1:
            ob = slice(off, out_hwm)
            oeng.dma_start(adv_d.ap()[hi, ob], adv[hi, ob])
            oeng.dma_start(tgt_d.ap()[hi, ob], tgt[hi, ob])
            out_hwm = off

    # tail: carry the boundary value across halves, fix up the early half.
    # Done in two column halves so the writeback DMA of the first overlaps
    # the fixup compute of the second.
    nc.sync.dma_start(bnd[:], adv[hi, 0:1])
    half = tp // 2
    for c0, c1 in ((0, half), (half, tp)):
        cs = slice(c0, c1)
        nc.vector.scalar_tensor_tensor(
            out=adv[lo, cs], in0=Pt[lo, cs], scalar=bnd[:, 0:1],
            in1=adv[lo, cs], op0=mult, op1=add,
        )
        nc.vector.scalar_tensor_tensor(
            out=tgt[lo, cs], in0=adv[lo, cs], scalar=b_t[:BL, 0:1],
            in1=v_raw[lo, cs], op0=add, op1=add,
        )
        nc.sync.dma_start(adv_d.ap()[lo, cs], adv[lo, cs])
        nc.scalar.dma_start(tgt_d.ap()[lo, cs], tgt[lo, cs])


def build_program(
    t_total=T, bulk=64, tail_sizes=(32, 16, 8, 8), repeat=1,
    in_dt=None, red_mode="tree", nv_on="dve", small_on="pool",
    bufs=4, dbl=2, bench_internal=False,
):
    """Build the per-core Bass program (all 8 cores run it SPMD on their own
    shard). DRAM tensor layouts are (half, batch)-major as produced by
    shard_inputs. repeat>1 re-runs the whole pipeline inside one NEFF
    (test.py uses the delta vs repeat=1 to measure per-iteration HW time).
    bench_internal makes obs/next_obs Internal DRAM (not shipped per call;
    garbage values) so benchmark calls are cheap — timing-only builds."""
    tp = t_total // H  # timesteps per partition
    rest = tp - sum(tail_sizes)
    assert rest >= 0 and rest % bulk == 0
    sizes = [bulk] * (rest // bulk) + list(tail_sizes)
    assert sum(sizes) == tp

    nc = bacc.Bacc(
        "TRN2", target_bir_lowering=False, debug=False, enable_asserts=False
    )

    if in_dt is None:
        in_dt = IN_DT
    sdt = BF16 if in_dt == "bf16" else F32
    big_kind = "Internal" if bench_internal else "ExternalInput"
    obs_d = nc.dram_tensor("obs", [P, tp * D], sdt, kind=big_kind)
    nobs_d = nc.dram_tensor("nobs", [P, tp * D], sdt, kind=big_kind)
    rw_d = nc.dram_tensor("rw", [P, tp], F32, kind="ExternalInput")
    dn_d = nc.dram_tensor("dn", [P, tp], U8, kind="ExternalInput")
    w_d = nc.dram_tensor("w", [D], sdt, kind="ExternalInput")
    b_d = nc.dram_tensor("b", [1], F32, kind="ExternalInput")
    adv_d = nc.dram_tensor("adv", [P, tp], F32, kind="ExternalOutput")
    tgt_d = nc.dram_tensor("tgt", [P, tp], F32, kind="ExternalOutput")

    with tile.TileContext(nc) as tc, ExitStack() as ctx:
        cpool = ctx.enter_context(tc.tile_pool(name="const", bufs=1))
        opool = ctx.enter_context(tc.tile_pool(name="obs", bufs=bufs))
        npool = ctx.enter_context(tc.tile_pool(name="nobs", bufs=bufs))
        qpool = ctx.enter_context(tc.tile_pool(name="chunk", bufs=3))
        dpool = ctx.enter_context(tc.tile_pool(name="iter", bufs=dbl))

        # Value-head weights replicated to every partition.
        w_t = cpool.tile([P, D], sdt)
        nc.sync.dma_start(w_t[:], w_d.ap().unsqueeze(0).broadcast_to([P, D]))
        b_t = cpool.tile([P, 1], F32)
        nc.sync.dma_start(b_t[:], b_d.ap().unsqueeze(0).broadcast_to([P, 1]))

        with nc.allow_low_precision("bf16 value head; tolerance is 2e-2"):
            for _rep in range(repeat):
                _build_iter2(
                    nc, opool, npool, qpool, dpool, w_t, b_t,
                    obs_d, nobs_d, rw_d, dn_d, adv_d, tgt_d, tp, sizes,
                    sdt, red_mode=red_mode, nv_on=nv_on, small_on=small_on,
                )

    # Runs the bacc pipeline (register allocation etc.) — required before
    # serializing for the walrus compiler.
    nc.finalize()
    return nc


_NC_CACHE = None


def _get_nc():
    global _NC_CACHE
    if _NC_CACHE is None:
        _NC_CACHE = build_program()
    return _NC_CACHE


def _hmajor(x, tp_cols):
    """[BL, H*tp_cols] row-major -> [H*BL, tp_cols] with row p = h*BL + b."""
    return np.ascontiguousarray(
        x.reshape(BL, H, tp_cols).transpose(1, 0, 2).reshape(H * BL, tp_cols)
    )


def _unhmajor(y):
    """Inverse of _hmajor for outputs: [H*BL, tp] -> [BL, H*tp]."""
    tp = y.shape[1]
    return y.reshape(H, BL, tp).transpose(1, 0, 2).reshape(BL, H * tp)


def shard_inputs(obs, next_obs, reward, done, W, b):
    """Split full inputs into the 8 per-core input maps ((h,b)-major)."""
    obs = np.asarray(obs, dtype=IN_NP).reshape(B, T * D)
    nobs = np.asarray(next_obs, dtype=IN_NP).reshape(B, T * D)
    rw = np.asarray(reward, dtype=np.float32).reshape(B, T)
    dn = np.asarray(done).astype(np.uint8, copy=False).reshape(B, T)
    w_np = np.ascontiguousarray(np.asarray(W, dtype=IN_NP)).reshape(D)
    b_np = np.ascontiguousarray(np.asarray(b, dtype=np.float32)).reshape(1)

    tpd = (T // H) * D
    tp = T // H
    in_maps = []
    for i in range(NCORES):
        sl = slice(i * BL, (i + 1) * BL)
        in_maps.append(
            {
                "obs": _hmajor(obs[sl], tpd),
                "nobs": _hmajor(nobs[sl], tpd),
                "rw": _hmajor(rw[sl], tp),
                "dn": _hmajor(dn[sl], tp),
                "w": w_np,
                "b": b_np,
            }
        )
    return in_maps


def gather_outputs(results):
    advantage = np.concatenate(
        [_unhmajor(r["adv"]) for r in results], axis=0
    ).reshape(B, T, 1)
    value_target = np.concatenate(
        [_unhmajor(r["tgt"]) for r in results], axis=0
    ).reshape(B, T, 1)
    return advantage, value_target


def kernel(obs, next_obs, reward, done, W, b):
    global LAST_RESULTS
    nc = _get_nc()
    in_maps = shard_inputs(obs, next_obs, reward, done, W, b)
    res = run_bass_kernel_spmd(nc, in_maps, core_ids=list(range(NCORES)))
    LAST_RESULTS = res
    return gather_outputs(res.results)


# revision 17
# speedup vs baseline: 2.5238x; 1.0980x over previous
"""GAE (generalized advantage estimation) Trainium2 kernel.

Problem: nn_CustomGAE — B=512, T=2048, D=64.
  value = obs @ W + b ; next_value = next_obs @ W + b
  td0 = reward + gamma*nd*next_value - value ; coef = gamma*lambda*nd
  A_t = td0_t + coef_t * A_{t+1}  (reverse scan over T, independent per trajectory)
  returns (advantage, value_target = advantage + value)

Sharding: pure data parallel over B across 8 cores (64 trajectories/core).

Per-core layout: the host pre-swizzles each 64-trajectory shard to
(half, batch)-major, so SBUF partition p = h*64 + b holds timesteps
t in [h*1024, (h+1)*1024) at a uniform DRAM stride — every streamed chunk is
one 128-partition dma_start with contiguous per-partition runs.

v2 pipeline (default):
  - obs streams on the SP HWDGE ring, next_obs on the Activation ring, so
    neither stream serializes behind the other; outputs go on the DVE ring.
  - DVE does obs*W + v-reduce; Pool (gpsimd) does next_obs*W + nv-reduce,
    so no single engine exceeds the DMA-engine-pool time per chunk.
  - Chunks stream in REVERSE time order with descending sizes at the end.
    Each chunk's td0 is computed on arrival (fused scalar_tensor_tensor) and
    scanned immediately, chaining `initial` across chunks. The late-time half
    (partitions 64..127) is exact and its outputs stream out per chunk; the
    early-time half scans with initial=0 while also scanning the running
    coef product P, and is fixed up at the very end as A = A0 + P*bnd
    (bnd = carry across the half boundary, one tiny SBUF->SBUF DMA).
    This leaves only a few microseconds of serial tail after the last byte
    arrives instead of the full epilogue+scan+writeback.
"""

import sys

sys.path.insert(0, "/opt/trn_rl_repo")

from contextlib import ExitStack

import numpy as np

import concourse.bacc as bacc
import concourse.mybir as mybir
import concourse.tile as tile
import ml_dtypes
from concourse.bass_utils import run_bass_kernel_spmd

GAMMA = 0.99
LMBDA = 0.95

B, T, D = 512, 2048, 64
NCORES = 8
BL = B // NCORES  # 64 trajectories per core
H = 2  # trajectory halves stacked on partitions -> 128 partitions
P = H * BL  # 128
F32 = mybir.dt.float32
BF16 = mybir.dt.bfloat16
U8 = mybir.dt.uint8

# obs/next_obs stream dtype: host casts to bf16 (value head is far inside the
# 2e-2 tolerance) which halves stream DMA and enables DVE packed-dtype modes.
IN_DT = "bf16"
IN_NP = ml_dtypes.bfloat16 if IN_DT == "bf16" else np.float32

# Results of the last hardware run, for test harnesses.
LAST_RESULTS = None


def _rsl(off, sz):
    """Reversed free-dim slice covering columns [off, off+sz)."""
    return slice(off + sz - 1, (off - 1) if off > 0 else None, -1)


def _build_iter2(
    nc, opool, qpool, dpool, w_t, b_t,
    obs_d, nobs_d, rw_d, dn_d, adv_d, tgt_d, tp, sizes,
    sdt, red_mode="tree", fold_w=16, nv_on="dve", small_on="pool",
):
    """One full pass, reverse-streamed with incremental blocked scan."""
    mult = mybir.AluOpType.mult
    add = mybir.AluOpType.add
    sub = mybir.AluOpType.subtract
    bypass = mybir.AluOpType.bypass
    X = mybir.AxisListType.X
    oeng = nc.scalar  # output DMAs ride the Activation ring between inputs

    rw_t = dpool.tile([P, tp], F32)
    dn_t = dpool.tile([P, tp], U8)
    nc.sync.dma_start(rw_t[:], rw_d.ap())
    nc.sync.dma_start(dn_t[:], dn_d.ap())

    # done -> decay factors, on Pool so the DMA rings stay compute-free.
    ndf = dpool.tile([P, tp], F32)
    nc.gpsimd.tensor_copy(ndf[:], dn_t[:])  # u8 -> f32
    g = dpool.tile([P, tp], F32)  # gamma * nd
    nc.gpsimd.tensor_scalar(g[:], ndf[:], -GAMMA, GAMMA, mult, add)
    coef = dpool.tile([P, tp], F32)  # gamma * lambda * nd
    nc.gpsimd.tensor_scalar(coef[:], ndf[:], -GAMMA * LMBDA, GAMMA * LMBDA, mult, add)
    # rw2 = rw + b*(g-1) folds the value-head bias out of the per-chunk path:
    # td0 = rw + g*(nv+b) - (v+b) = (g*nv - v) + rw2, all plain tensor_tensor.
    # (g-1 overwrites ndf, rw2 overwrites rw in place to save SBUF.)
    nc.gpsimd.tensor_scalar(ndf[:], ndf[:], -GAMMA, GAMMA - 1.0, mult, add)
    rw2 = rw_t
    nc.vector.scalar_tensor_tensor(
        out=rw2[:], in0=ndf[:], scalar=b_t[:, 0:1], in1=rw_t[:], op0=mult, op1=add,
    )

    vdt = sdt if red_mode == "direct" else F32
    v_raw = dpool.tile([P, tp], vdt)  # obs @ W (no bias)
    adv = dpool.tile([P, tp], F32)
    Pt = dpool.tile([P, tp], F32)  # running suffix product of coef
    tgt = dpool.tile([P, tp], F32)
    bnd = dpool.tile([BL, 1], F32)

    hi = slice(BL, 2 * BL)
    lo = slice(0, BL)
    bulk = max(sizes)
    wb = {
        sz: w_t[:].unsqueeze(1).broadcast_to([P, 2 * sz, D])
        for sz in sorted(set(sizes))
    }

    off = tp
    prev_off = None  # column of the previously scanned chunk's first element
    out_hwm = tp  # columns [out_hwm, tp) already written back (hi half)
    for j, sz in enumerate(sizes):
        off -= sz
        cs = slice(off, off + sz)
        fs = slice(off * D, (off + sz) * D)
        # obs chunk lands in the first half of the tile, next_obs (other DMA
        # ring) in the second, so the value-head mult and the halving folds
        # cover both streams in single instructions.
        ot = opool.tile([P, 2 * bulk * D], sdt)
        nc.sync.dma_start(ot[:, : sz * D], obs_d.ap()[:, fs])
        nc.scalar.dma_start(ot[:, sz * D : 2 * sz * D], nobs_d.ap()[:, fs])
        x3 = ot[:, : 2 * sz * D].rearrange("p (t d) -> p t d", d=D)

        # value head: in-place mult, then fold D down in the 2-byte packed
        # fast mode (TensorReduce has no packed mode; free-axis reduces are
        # DVE-only and Pool is ~3x slower per element on this HW)
        nvs = qpool.tile([P, bulk], vdt)
        nc.vector.tensor_tensor(out=x3, in0=x3, in1=wb[sz], op=mult)
        if red_mode == "direct":
            nc.vector.tensor_reduce(out=v_raw[:, cs], in_=x3[:, :sz], axis=X, op=add)
            nc.vector.tensor_reduce(out=nvs[:, :sz], in_=x3[:, sz:], axis=X, op=add)
        else:
            w2 = D // 2
            while w2 >= fold_w:
                nc.vector.tensor_tensor(
                    out=x3[:, :, :w2], in0=x3[:, :, :w2],
                    in1=x3[:, :, w2 : 2 * w2], op=add,
                )
                w2 //= 2
            nc.vector.tensor_reduce(
                out=v_raw[:, cs], in_=x3[:, :sz, : 2 * w2], axis=X, op=add
            )
            nc.vector.tensor_reduce(
                out=nvs[:, :sz], in_=x3[:, sz:, : 2 * w2], axis=X, op=add
            )
        seng = nc.gpsimd if small_on == "pool" else nc.vector

        # td0 = g*nv - v + rw2, all Pool-eligible tensor_tensor ops
        q = qpool.tile([P, bulk], F32)
        seng.tensor_tensor(out=q[:, :sz], in0=g[:, cs], in1=nvs[:, :sz], op=mult)
        t1 = qpool.tile([P, bulk], F32)
        seng.tensor_tensor(out=t1[:, :sz], in0=rw2[:, cs], in1=v_raw[:, cs], op=sub)
        seng.tensor_tensor(out=q[:, :sz], in0=q[:, :sz], in1=t1[:, :sz], op=add)

        # incremental reverse scans, chained via `initial`
        rs = _rsl(off, sz)
        qrs = slice(sz - 1, None, -1)
        a_init = 0.0 if prev_off is None else adv[:, prev_off : prev_off + 1]
        nc.vector.tensor_tensor_scan(
            out=adv[:, rs], data0=coef[:, rs], data1=q[:, qrs],
            initial=a_init, op0=mult, op1=add,
        )
        p_init = 1.0 if prev_off is None else Pt[:, prev_off : prev_off + 1]
        nc.vector.tensor_tensor_scan(
            out=Pt[:, rs], data0=coef[:, rs], data1=coef[:, rs],
            initial=p_init, op0=mult, op1=bypass,
        )
        prev_off = off

        # late-time half is exact: finalize per chunk, stream out in batches
        nc.vector.scalar_tensor_tensor(
            out=tgt[hi, cs], in0=adv[hi, cs], scalar=b_t[BL:, 0:1],
            in1=v_raw[hi, cs], op0=add, op1=add,
        )
        if out_hwm - off >= 192 or j == len(sizes) - 1:
            ob = slice(off, out_hwm)
            oeng.dma_start(adv_d.ap()[hi, ob], adv[hi, ob])
            oeng.dma_start(tgt_d.ap()[hi, ob], tgt[hi, ob])
            out_hwm = off

    # tail: carry the boundary value across halves, fix up the early half.
    # Done in two column halves so the writeback DMA of the first overlaps
    # the fixup compute of the second.
    nc.sync.dma_start(bnd[:], adv[hi, 0:1])
    half = tp // 2
    for c0, c1 in ((0, half), (half, tp)):
        cs = slice(c0, c1)
        nc.vector.scalar_tensor_tensor(
            out=adv[lo, cs], in0=Pt[lo, cs], scalar=bnd[:, 0:1],
            in1=adv[lo, cs], op0=mult, op1=add,
        )
        nc.vector.scalar_tensor_tensor(
            out=tgt[lo, cs], in0=adv[lo, cs], scalar=b_t[:BL, 0:1],
            in1=v_raw[lo, cs], op0=add, op1=add,
        )
        nc.sync.dma_start(adv_d.ap()[lo, cs], adv[lo, cs])
        nc.scalar.dma_start(tgt_d.ap()[lo, cs], tgt[lo, cs])


def build_program(
    t_total=T, bulk=128, head_sizes=(32, 96), tail_sizes=(64, 32, 16, 8, 8),
    repeat=1,
    in_dt=None, red_mode="tree", fold_w=16, nv_on="dve", small_on="pool",
    bufs=3, dbl=2, bench_internal=False,
):
    """Build the per-core Bass program (all 8 cores run it SPMD on their own
    shard). DRAM tensor layouts are (half, batch)-major as produced by
    shard_inputs. repeat>1 re-runs the whole pipeline inside one NEFF
    (test.py uses the delta vs repeat=1 to measure per-iteration HW time).
    bench_internal makes obs/next_obs Internal DRAM (not shipped per call;
    garbage values) so benchmark calls are cheap — timing-only builds."""
    tp = t_total // H  # timesteps per partition
    rest = tp - sum(tail_sizes) - sum(head_sizes)
    assert rest >= 0 and rest % bulk == 0
    sizes = list(head_sizes) + [bulk] * (rest // bulk) + list(tail_sizes)
    assert sum(sizes) == tp

    nc = bacc.Bacc(
        "TRN2", target_bir_lowering=False, debug=False, enable_asserts=False
    )

    if in_dt is None:
        in_dt = IN_DT
    sdt = BF16 if in_dt == "bf16" else F32
    big_kind = "Internal" if bench_internal else "ExternalInput"
    obs_d = nc.dram_tensor("obs", [P, tp * D], sdt, kind=big_kind)
    nobs_d = nc.dram_tensor("nobs", [P, tp * D], sdt, kind=big_kind)
    rw_d = nc.dram_tensor("rw", [P, tp], F32, kind="ExternalInput")
    dn_d = nc.dram_tensor("dn", [P, tp], U8, kind="ExternalInput")
    w_d = nc.dram_tensor("w", [D], sdt, kind="ExternalInput")
    b_d = nc.dram_tensor("b", [1], F32, kind="ExternalInput")
    adv_d = nc.dram_tensor("adv", [P, tp], F32, kind="ExternalOutput")
    tgt_d = nc.dram_tensor("tgt", [P, tp], F32, kind="ExternalOutput")

    with tile.TileContext(nc) as tc, ExitStack() as ctx:
        cpool = ctx.enter_context(tc.tile_pool(name="const", bufs=1))
        opool = ctx.enter_context(tc.tile_pool(name="obs", bufs=bufs))
        qpool = ctx.enter_context(tc.tile_pool(name="chunk", bufs=3))
        dpool = ctx.enter_context(tc.tile_pool(name="iter", bufs=dbl))

        # Value-head weights replicated to every partition.
        w_t = cpool.tile([P, D], sdt)
        nc.sync.dma_start(w_t[:], w_d.ap().unsqueeze(0).broadcast_to([P, D]))
        b_t = cpool.tile([P, 1], F32)
        nc.sync.dma_start(b_t[:], b_d.ap().unsqueeze(0).broadcast_to([P, 1]))

        with nc.allow_low_precision("bf16 value head; tolerance is 2e-2"):
            for _rep in range(repeat):
                _build_iter2(
                    nc, opool, qpool, dpool, w_t, b_t,
                    obs_d, nobs_d, rw_d, dn_d, adv_d, tgt_d, tp, sizes,
                    sdt, red_mode=red_mode, fold_w=fold_w, nv_on=nv_on,
                    small_on=small_on,
                )

    # Runs the bacc pipeline (register allocation etc.) — required before
    # serializing for the walrus compiler.
    nc.finalize()
    return nc


_NC_CACHE = None


def _get_nc():
    global _NC_CACHE
    if _NC_CACHE is None:
        _NC_CACHE = build_program()
    return _NC_CACHE


def _hmajor(x, tp_cols):
    """[BL, H*tp_cols] row-major -> [H*BL, tp_cols] with row p = h*BL + b."""
    return np.ascontiguousarray(
        x.reshape(BL, H, tp_cols).transpose(1, 0, 2).reshape(H * BL, tp_cols)
    )


def _unhmajor(y):
    """Inverse of _hmajor for outputs: [H*BL, tp] -> [BL, H*tp]."""
    tp = y.shape[1]
    return y.reshape(H, BL, tp).transpose(1, 0, 2).reshape(BL, H * tp)


def shard_inputs(obs, next_obs, reward, done, W, b):
    """Split full inputs into the 8 per-core input maps ((h,b)-major)."""
    obs = np.asarray(obs, dtype=IN_NP).reshape(B, T * D)
    nobs = np.asarray(next_obs, dtype=IN_NP).reshape(B, T * D)
    rw = np.asarray(reward, dtype=np.float32).reshape(B, T)
    dn = np.asarray(done).astype(np.uint8, copy=False).reshape(B, T)
    w_np = np.ascontiguousarray(np.asarray(W, dtype=IN_NP)).reshape(D)
    b_np = np.ascontiguousarray(np.asarray(b, dtype=np.float32)).reshape(1)

    tpd = (T // H) * D
    tp = T // H
    in_maps = []
    for i in range(NCORES):
        sl = slice(i * BL, (i + 1) * BL)
        in_maps.append(
            {
                "obs": _hmajor(obs[sl], tpd),
                "nobs": _hmajor(nobs[sl], tpd),
                "rw": _hmajor(rw[sl], tp),
                "dn": _hmajor(dn[sl], tp),
                "w": w_np,
                "b": b_np,
            }
        )
    return in_maps


def gather_outputs(results):
    advantage = np.concatenate(
        [_unhmajor(r["adv"]) for r in results], axis=0
    ).reshape(B, T, 1)
    value_target = np.concatenate(
        [_unhmajor(r["tgt"]) for r in results], axis=0
    ).reshape(B, T, 1)
    return advantage, value_target


def kernel(obs, next_obs, reward, done, W, b):
    global LAST_RESULTS
    nc = _get_nc()
    in_maps = shard_inputs(obs, next_obs, reward, done, W, b)
    res = run_bass_kernel_spmd(nc, in_maps, core_ids=list(range(NCORES)))
    LAST_RESULTS = res
    return gather_outputs(res.results)
